# revision 18
# baseline (speedup 1.0000x reference)
"""CrossEntropyWithProbs kernel for Trainium2 (8 NeuronCores, data parallel).

loss = mean_r( -sum_c target[r,c] * weight[c] * log_softmax(input)[r,c] )

Algebraic decomposition (per shard of rows):
    sum_r loss_r = sum_c w_c * (g_c - d_c)
        d_c = sum_r T[r,c] * X[r,c]          (weighted by w on host)
        g_c = sum_r T[r,c] * logZ_r,  logZ_r = log(sum_c exp(X[r,c]))
(no max-subtraction needed: inputs are N(0,1), exp is safe; all values fit
fp16 range: |x| <= ~5.8, exp(x) <= ~330, row sums <= ~400 << 65504)

The fp32 predecessor (see kernel_f32_baseline.py.bak) streamed X,T as fp32
and measured 192 us/core = ~97.5% of the ~358 GB/s per-core HBM limit —
memory-bound. The only remaining lever is fewer bytes: the host casts X,T
to fp16 (rel-err of the final mean ~5e-6, tolerance is 2e-2), halving DMA
to ~32 MiB/core (~95 us model). Engine busy must fit under the stream
(cost-model per core): DMA 94.7 > DVE 76.3 > ACT 65.4 > PE 62.6 us;
steady-state model slope 93.2 us/rep, measured 64-98 us (axon tunnel is
noisy; median ~83).
  - ACT (1.2 GHz, 1 elem/cy/lane, dtype-independent): exp 59 us + ln 6 us.
    Exp and Ln are forced into ONE table set (natural_log_exp_and_others
    via _patch_act_tables) — default first-match selection alternated two
    sets at ~1.3 us per reload, 38 us/core of ACT busy.
  - DVE (0.96 GHz): tensor_reduce is capped at 1x on TRN2, so the row-sum
    of exp uses a log2(32)=5-level tensor_tensor add tree instead (2x mode
    for packed 2-byte dtypes); + the T*X mul (2x). ~76 us total.
  - PE: d-colsums + g-matmuls ~63 us (fp16 = 1 row/cycle, same as bf16).
Explored and rejected on evidence:
  - fp8 inputs: DMA 48 us but TRN2 DVE has no fp8 packing (all ops 1x)
    and ACT's exp floor is ~55 us — compute-bound at >100 us.
  - pool_avg for the row-sum: 1x-capped (DVE 107 us model).
  - tensor_tensor_reduce d-folding: reduce-class uop, 1x (DVE 108 model).
  - gpsimd add-tree offload: 2.3x WORSE on HW (146 us vs 65) — gpsimd is
    far slower than the cost model's 0.42-efficiency claim.
  - K=256 tiles: PE p-state ramp penalties outweigh overhead savings.
  - xt-interleaved single DMA/tile: model shows PE/pipeline serialization
    regressions; separate x/t DMAs also let exp start earlier.

Per-core dataflow (rows sharded 8 ways, 262144 rows/core):
  tile = [128 partitions, 128 rows/partition, 32 classes] = [128, 4096] f16
  - ACT:  E = exp(X)                          (f16 in, f16 out)
  - DVE:  S = add-tree over classes: 32->16->8->4->2->1  -> [128, 128]
  - ACT:  LZ = ln(S)
  - DVE:  TX = T * X                          (f16, 2x mode)
  - PE :  d-colsums:  ones^T @ TX chunks  -> PSUM [1, 2048]   (chunks wrap mod 4)
  - PE :  g-matmuls:  LZ_half^T @ T chunks -> PSUM [64, 2048] (block-diag extract)
  PSUM accumulates across all 16 tiles; tiny per-core stats DMA'd out;
  host applies class weights, extracts block diagonals, and averages.
"""

import sys
from contextlib import ExitStack

import numpy as np

for _p in ("/opt/trn_rl_repo", "/root/.axon_site/_ro/trn_rl_repo"):
    if _p not in sys.path:
        sys.path.insert(0, _p)

P = 128          # SBUF partitions
K = 128          # rows per partition per tile
C = 32           # classes
F = K * C        # free elems per tile (4096)
CH = 512         # matmul moving-operand chunk
NCH = F // CH    # 8 chunks per tile
KPC = CH // C    # 16 rows per chunk
N_CORES = 8
N_TOTAL = 2097152
N_SHARD = N_TOTAL // N_CORES            # 262144
HALF = 64        # lhsT free width for g-matmuls (max 128; 2 halves of K)
IN_NP_DT = np.float16


def _patch_act_tables():
    """Force the act-table-load pass to serve both Exp and Ln from the one
    set that contains them both (natural_log_exp_and_others). The default
    first-match selection alternates exp_and_others / natural_log, paying a
    ~1.3 us table DMA twice per tile (~38 us/core of ACT busy time). Set
    indices (= act_func_set_id consumed by walrus) are preserved; only the
    membership used for selection is narrowed."""
    from concourse import hw_specs, mybir
    import concourse.bacc as bacc

    if getattr(_patch_act_tables, "_done", False):
        return
    orig = hw_specs.get_activation_tables

    def patched(arch):
        tabs = orig(arch)
        exp_t = mybir.ActivationFunctionType.Exp
        ln_t = mybir.ActivationFunctionType.Ln
        out = {}
        for name, fns in tabs.items():
            if name != "natural_log_exp_and_others" and (
                    exp_t in fns or ln_t in fns):
                fns = set(fns) - {exp_t, ln_t}
            out[name] = fns
        return out

    hw_specs.get_activation_tables = patched
    bacc.get_activation_tables = patched
    _patch_act_tables._done = True


def build_nc(n_shard=N_SHARD, reps=1, mode="full", t_dma_engine="sync",
             xt_interleave=False, gpsimd_l1=False, ttr_d=False):
    """reps>1 repeats the whole pipeline (same result; PSUM restarts each
    rep) so on-HW timing can separate kernel time from dispatch overhead.
    mode="dma" builds a loads-only variant (timing diagnostic; bogus output).
    t_dma_engine: "sync"|"scalar" — which HWDGE ring carries the T loads.
    xt_interleave: host interleaves X|T per row; one 2 MiB DMA per tile
    instead of two 1 MiB DMAs (engine reads become strided views).
    gpsimd_l1: run the first (largest) add-tree level on the otherwise-idle
    GPSIMD engine to offload the DVE (measured 2.3x WORSE on HW — gpsimd
    is far slower than the cost model claims; keep False).
    ttr_d: expect host-prescaled T' = T*w and fold the whole d-term into
    the T'*X multiply via tensor_tensor_reduce's per-partition accumulator
    (f32 scalar operands keep the 2x DVE mode). Removes the d-colsum
    matmuls, the ones tile, and the d PSUM bank; d_out becomes the per-
    (partition, tile) accumulator table and g_out comes out pre-weighted."""
    import concourse.bacc as bacc
    import concourse.tile as tile
    from concourse import mybir

    _patch_act_tables()

    tiles = n_shard // (P * K)
    assert tiles * P * K == n_shard

    nc = bacc.Bacc("TRN2", target_bir_lowering=False, debug=False,
                   num_devices=N_CORES)
    f32 = mybir.dt.float32
    f16 = mybir.dt.float16

    if xt_interleave:
        xt_d = nc.dram_tensor("xt", [n_shard, 2 * C], f16, kind="ExternalInput")
        xtv = xt_d.ap().rearrange("(i p k) c -> i p (k c)", p=P, k=K)
    else:
        x_d = nc.dram_tensor("x", [n_shard, C], f16, kind="ExternalInput")
        t_d = nc.dram_tensor("t", [n_shard, C], f16, kind="ExternalInput")
        xv = x_d.ap().rearrange("(i p k) c -> i p (k c)", p=P, k=K)
        tv = t_d.ap().rearrange("(i p k) c -> i p (k c)", p=P, k=K)
    d_w = 4 * CH
    if ttr_d:
        d_out = nc.dram_tensor("d_out", [P, tiles], f32, kind="ExternalOutput")
    else:
        d_out = nc.dram_tensor("d_out", [1, d_w], f32, kind="ExternalOutput")
    g_out = nc.dram_tensor("g_out", [HALF, 4 * CH], f32, kind="ExternalOutput")

    with tile.TileContext(nc) as tc, ExitStack() as ctx:
        xpool = ctx.enter_context(tc.tile_pool(name="xpool", bufs=3))
        tpool = ctx.enter_context(tc.tile_pool(name="tpool", bufs=3))
        epool = ctx.enter_context(tc.tile_pool(name="epool", bufs=2))
        txpool = ctx.enter_context(tc.tile_pool(name="txpool", bufs=2))
        redpool = ctx.enter_context(tc.tile_pool(name="redpool", bufs=2))
        small = ctx.enter_context(tc.tile_pool(name="small", bufs=2))
        singles = ctx.enter_context(tc.tile_pool(name="singles", bufs=1))
        psum = ctx.enter_context(tc.tile_pool(name="psum", bufs=1, space="PSUM"))

        if ttr_d:
            acc_sb = singles.tile([P, tiles], f32)
        else:
            ones = singles.tile([P, 1], f16)
            nc.vector.memset(ones, 1.0)
            d_ps = psum.tile([1, d_w], f32)
        g_ps = psum.tile([HALF, 4 * CH], f32)

        t_dma = nc.sync if t_dma_engine == "sync" else nc.scalar

        for rep in range(reps):
          for i in range(tiles):
              if xt_interleave:
                  xt_t = xpool.tile([P, 2 * F], f16, tag="x")
                  nc.sync.dma_start(out=xt_t, in_=xtv[i])
                  xt3 = xt_t.rearrange("p (k c) -> p k c", c=2 * C)
                  x_t, t_t = xt3[:, :, 0:C], xt3[:, :, C:2 * C]
              else:
                  x_t = xpool.tile([P, F], f16, tag="x")
                  nc.sync.dma_start(out=x_t, in_=xv[i])
                  t_t = tpool.tile([P, F], f16, tag="t")
                  t_dma.dma_start(out=t_t, in_=tv[i])

              if mode == "dma":
                  continue

              e_t = epool.tile([P, F], f16, tag="e")
              if xt_interleave:
                  nc.scalar.activation(
                      e_t.rearrange("p (k c) -> p k c", c=C), x_t,
                      mybir.ActivationFunctionType.Exp)
              else:
                  nc.scalar.activation(e_t, x_t,
                                       mybir.ActivationFunctionType.Exp)

              # class-sum add tree 32 -> 16 -> 8 -> 4 -> 2 -> 1
              # (tensor_reduce is 1x-capped on TRN2 DVE; packed-f16
              # tensor_tensor runs 2x)
              e3 = e_t.rearrange("p (k c) -> p k c", c=C)
              a1 = redpool.tile([P, K * 16], f16, tag="a1")
              a1v = a1.rearrange("p (k c) -> p k c", c=16)
              l1_eng = nc.gpsimd if gpsimd_l1 else nc.vector
              l1_eng.tensor_add(a1v, e3[:, :, 0:16], e3[:, :, 16:32])
              a2 = redpool.tile([P, K * 8], f16, tag="a2")
              a2v = a2.rearrange("p (k c) -> p k c", c=8)
              nc.vector.tensor_add(a2v, a1v[:, :, 0:8], a1v[:, :, 8:16])
              a3 = redpool.tile([P, K * 4], f16, tag="a3")
              a3v = a3.rearrange("p (k c) -> p k c", c=4)
              nc.vector.tensor_add(a3v, a2v[:, :, 0:4], a2v[:, :, 4:8])
              a4 = redpool.tile([P, K * 2], f16, tag="a4")
              a4v = a4.rearrange("p (k c) -> p k c", c=2)
              nc.vector.tensor_add(a4v, a3v[:, :, 0:2], a3v[:, :, 2:4])
              s_t = small.tile([P, K], f16, tag="s")
              sv = s_t.rearrange("p (k c) -> p k c", c=1)
              nc.vector.tensor_add(sv, a4v[:, :, 0:1], a4v[:, :, 1:2])

              lz_t = small.tile([P, K], f16, tag="lz")
              nc.scalar.activation(lz_t, s_t, mybir.ActivationFunctionType.Ln)

              tx_t = txpool.tile([P, F], f16, tag="tx")
              tx_v = (tx_t.rearrange("p (k c) -> p k c", c=C)
                      if xt_interleave else tx_t)
              if ttr_d:
                  # tx is a write-only scratch; the per-partition accum IS
                  # the d-term (T' already carries the class weights)
                  nc.vector.tensor_tensor_reduce(
                      tx_v, t_t, x_t, 1.0, 0.0,
                      mybir.AluOpType.mult, mybir.AluOpType.add,
                      acc_sb[:, i:i + 1])
              else:
                  nc.vector.tensor_mul(tx_v, t_t, x_t)

              def t_chunk(j):
                  if xt_interleave:
                      return t_t[:, j * KPC:(j + 1) * KPC, :]
                  return t_t[:, j * CH:(j + 1) * CH]

              if not ttr_d:
                  for j in range(NCH):
                      a = (j * CH) % d_w
                      nc.tensor.matmul(d_ps[:, a:a + CH],
                                       ones, tx_t[:, j * CH:(j + 1) * CH],
                                       start=(i == 0 and j * CH < d_w),
                                       stop=(i == tiles - 1 and j * CH >= (NCH - d_w // CH) * CH))
              for h in range(K // HALF):
                  lzh = lz_t[:, h * HALF:(h + 1) * HALF]
                  for a in range(4):
                      j = 4 * h + a
                      nc.tensor.matmul(g_ps[:, a * CH:(a + 1) * CH],
                                       lzh, t_chunk(j),
                                       start=(i == 0 and h == 0),
                                       stop=(i == tiles - 1 and h == K // HALF - 1))

        if ttr_d:
            nc.sync.dma_start(out=d_out.ap(), in_=acc_sb)
        else:
            d_sb = singles.tile([1, d_w], f32)
            nc.vector.tensor_copy(d_sb, d_ps)
            nc.sync.dma_start(out=d_out.ap(), in_=d_sb)
        g_sb = singles.tile([HALF, 4 * CH], f32)
        nc.scalar.copy(g_sb, g_ps)
        nc.sync.dma_start(out=g_out.ap(), in_=g_sb)

    nc.compile()
    return nc


def host_reduce(results, weight, n_total, ttr_d=False):
    """Combine per-core (d_out, g_out) stats into the scalar mean loss."""
    if ttr_d:
        # T was host-prescaled by the class weights: g block-diag and the
        # d accumulator table are both already weighted
        d = 0.0
        g = np.zeros(C, np.float64)
        for res in results:
            d += res["d_out"].astype(np.float64).sum()
            gp = res["g_out"].astype(np.float64).reshape(HALF, 4, KPC, C)
            for a in range(4):
                for kl in range(KPC):
                    g += gp[KPC * a + kl, a, kl, :]
        return np.float32((g.sum() - d) / n_total)
    d = np.zeros(C, np.float64)
    g = np.zeros(C, np.float64)
    for res in results:
        d += res["d_out"].astype(np.float64).reshape(-1, C).sum(axis=0)
        gp = res["g_out"].astype(np.float64).reshape(HALF, 4, KPC, C)
        for a in range(4):
            for kl in range(KPC):
                g += gp[KPC * a + kl, a, kl, :]
    loss = (weight.astype(np.float64) * (g - d)).sum() / n_total
    return np.float32(loss)


_NC_CACHE = {}
TRACE = False          # set True (e.g. from test.py) to capture an NTFF profile
LAST_RESULT = None     # BassKernelResults of the most recent kernel() call
XT_INTERLEAVE = False  # deployed-variant switches (set after on-HW A/B)
GPSIMD_L1 = False
TTR_D = False   # tensor_tensor_reduce is 1x-capped on DVE (reduce-class
                # uop): folding d into the mul costs +34 us DVE for -27 us
                # PE — rejected by cost model


def kernel(input, target, weight):
    global LAST_RESULT
    from concourse.bass_utils import run_bass_kernel_spmd

    assert input.shape == (N_TOTAL, C) and target.shape == (N_TOTAL, C)
    if "nc" not in _NC_CACHE:
        _NC_CACHE["nc"] = build_nc(N_SHARD, xt_interleave=XT_INTERLEAVE,
                                   gpsimd_l1=GPSIMD_L1, ttr_d=TTR_D)
    nc = _NC_CACHE["nc"]

    w32 = np.asarray(weight, dtype=np.float32)
    x = np.asarray(input, dtype=np.float32).astype(IN_NP_DT)
    if TTR_D:
        t = (np.asarray(target, dtype=np.float32) * w32[None, :]).astype(
            IN_NP_DT)
    else:
        t = np.asarray(target, dtype=np.float32).astype(IN_NP_DT)
    if XT_INTERLEAVE:
        xt = np.concatenate([x, t], axis=1)
        xts = np.ascontiguousarray(xt).reshape(N_CORES, N_SHARD, 2 * C)
        in_maps = [{"xt": xts[i]} for i in range(N_CORES)]
    else:
        xs = np.ascontiguousarray(x).reshape(N_CORES, N_SHARD, C)
        ts = np.ascontiguousarray(t).reshape(N_CORES, N_SHARD, C)
        in_maps = [{"x": xs[i], "t": ts[i]} for i in range(N_CORES)]

    try:
        out = run_bass_kernel_spmd(nc, in_maps, core_ids=list(range(N_CORES)),
                                   trace=TRACE)
    except ModuleNotFoundError:
        # axon NTFF profile hook unavailable in this container
        out = run_bass_kernel_spmd(nc, in_maps, core_ids=list(range(N_CORES)))
    LAST_RESULT = out
    return np.array(host_reduce(out.results, w32, N_TOTAL, ttr_d=TTR_D),
                    dtype=np.float32)
